# revision 4
# baseline (speedup 1.0000x reference)
"""Trainium2 Bass kernel for im2col Conv2d dot-product, PE (tensor engine)
version: out[b, n] = <enc_x[b, n, :], w_flat> + bias.

Data-parallel over batch: 8 batches per NeuronCore x 8 cores.

The host casts to fp16 (tolerance 2e-2; fp16 keeps rel err ~1e-4, and PE
accumulates in fp32 PSUM) and lays x out k-major: per core the flat
window stream [401408*49] is reshaped to [12544, 1568] and transposed to
[1568, 12544], so column c holds the 1568 contiguous values of windows
32c..32c+31; a 1569th row of ones carries the bias. The stationary
matrix S [1569, 32] is block-diagonal w (S[r, r//49] = w[r%49], ones row
= bias), split into 13 partition blocks of <=128 rows. Each output chunk
[32, 448] is produced by 13 accumulating matmuls (start on h=0, stop on
h=12), evacuated PSUM->SBUF by the ScalarE, and DMAed out as [32, 12544]
which the host transposes back to window order.

PE streams 448 cols/matmul at ~1 cycle/col (fp16) -> ~70-90 us/core of
matmul; DVE and GpSimd are idle; the kernel is DMA-bound (~39.7 MB/core
at ~340 GB/s -> ~120 us).
"""

from contextlib import ExitStack

import numpy as np

import concourse.bass as bass
import concourse.tile as tile
from concourse import mybir

B = 64
WINDOWS = 50176
K = 49
NCORES = 8
BPC = B // NCORES            # batches per core
NWIN = BPC * WINDOWS         # 401408 windows per core
G = 32                       # windows per column
ROWS = G * K                 # 1568 data rows
NBLK = (ROWS + 1 + 127) // 128   # 13 partition blocks (incl. ones row)
COLS = NWIN // G             # 12544 columns per core

CHUNK = 448                  # columns per PSUM chunk (32*448*4 = 1792B/bank)
# Variable column-tile schedule: big tiles early (fat DMAs while the
# pipeline is deep), small final tile so the post-last-DMA drain
# (h=12 matmul + PSUM evacuation + out-DMA) is short.
CTS = [3136, 3136, 3136, 1792, 896, 448]
assert sum(CTS) == COLS and all(c % CHUNK == 0 for c in CTS)
CTMAX = max(CTS)

FP32 = mybir.dt.float32
FP16 = mybir.dt.float16

_NC = None


def _build_nc():
    nc = bass.Bass(trn_type="TRN2", debug=False, num_devices=NCORES)

    x = nc.dram_tensor("x", [ROWS + 1, COLS], FP16, kind="ExternalInput").ap()
    # padded to 128*NBLK rows so one DMA loads all stationary blocks
    s = nc.dram_tensor("s", [128 * NBLK, G], FP16, kind="ExternalInput").ap()
    out = nc.dram_tensor("out", [G, COLS], FP16, kind="ExternalOutput").ap()

    blk_rows = [min(128, ROWS + 1 - 128 * h) for h in range(NBLK)]

    with tile.TileContext(nc) as tc, ExitStack() as ctx:
        consts = ctx.enter_context(tc.tile_pool(name="consts", bufs=1))
        xpool = ctx.enter_context(tc.tile_pool(name="x", bufs=2))
        apool = ctx.enter_context(tc.tile_pool(name="a", bufs=2))
        ppool = ctx.enter_context(tc.psum_pool(name="p", bufs=8))

        # all stationary blocks in one tile [128, NBLK, G], one DMA
        sblk = consts.tile([128, NBLK, G], FP16, name="sblk")
        nc.gpsimd.dma_start(
            out=sblk[:],
            in_=bass.AP(
                tensor=s.tensor,
                offset=s.offset,
                ap=[[G, 128], [128 * G, NBLK], [1, G]],
            ),
        )
        sts = [sblk[:][0 : blk_rows[h], h, :] for h in range(NBLK)]

        c0 = 0
        for ct, CT in enumerate(CTS):
            nchunk = CT // CHUNK
            # fixed-shape ring buffers; only the first CT columns are used
            xb = xpool.tile([128, NBLK, CTMAX], FP16, tag="xb", name=f"xb{ct}")
            xv = xb[:]
            for h in range(NBLK):
                src = bass.AP(
                    tensor=x.tensor,
                    offset=x.offset + (128 * h) * COLS + c0,
                    ap=[[COLS, blk_rows[h]], [1, CT]],
                )
                nc.sync.dma_start(out=xv[0 : blk_rows[h], h, 0:CT], in_=src)

            acc = apool.tile(
                [G, CTMAX // CHUNK, CHUNK], FP16, tag="acc", name=f"acc{ct}"
            )
            # h-outer: consecutive matmuls share the stationary block, so
            # walrus can skip redundant LDWEIGHTS; chunks accumulate in
            # parallel PSUM banks (groups interleave -> skip_group_check).
            pts = [
                ppool.tile([G, CHUNK], FP32, tag="pt", name=f"pt{ct}_{ci}")
                for ci in range(nchunk)
            ]
            for h in range(NBLK):
                for ci in range(nchunk):
                    nc.tensor.matmul(
                        pts[ci][:],
                        sts[h],
                        xv[0 : blk_rows[h], h, ci * CHUNK : (ci + 1) * CHUNK],
                        start=(h == 0),
                        stop=(h == NBLK - 1),
                        skip_group_check=True,
                    )
            for ci in range(nchunk):
                nc.scalar.copy(out=acc[:, ci, :], in_=pts[ci][:])

            dst = bass.AP(
                tensor=out.tensor,
                offset=out.offset + c0,
                ap=[[COLS, G], [1, CT]],
            )
            nc.scalar.dma_start(
                out=dst,
                in_=acc[:, 0:nchunk, :].rearrange("g c k -> g (c k)"),
            )
            c0 += CT
        assert c0 == COLS

    return nc


def _split_ctrl_waits(nc, max_waits=1):
    """Work around a walrus codegen limit on this build: instructions accept
    only one sync-wait command. Hoist extra waits onto dedicated no-op
    instructions inserted just before, preserving per-engine order."""
    from concourse import mybir

    for f in nc.m.functions:
        for blk in f.blocks:
            insts = blk.instructions
            i = 0
            while i < len(insts):
                ins = insts[i]
                if (
                    ins.sync_info is not None
                    and len(ins.sync_info.on_wait) > max_waits
                ):
                    waits = list(ins.sync_info.on_wait)
                    keep, extra = waits[:max_waits], waits[max_waits:]
                    ins.sync_info.on_wait = keep
                    for j, wchunk in enumerate(extra):
                        nop = mybir.InstNoOp(
                            name=f"{ins.name}-wsplit{j}",
                            sync_info=mybir.SyncInfo(on_wait=[wchunk], on_update=[]),
                            bass_nofuse=True,
                            engine=ins.engine,
                        )
                        nc.register_instruction(nop, overwrite=True)
                        insts.insert(i, nop)
                        i += 1
                i += 1


def _get_nc():
    global _NC
    if _NC is None:
        _NC = _build_nc()
        _split_ctrl_waits(_NC)
    return _NC


def _prep_core(xc16):
    """[NWIN, K] fp16 -> [ROWS+1, COLS] fp16 k-major layout + ones row."""
    xt = np.empty((ROWS + 1, COLS), dtype=np.float16)
    xt[:ROWS] = xc16.reshape(COLS, ROWS).T
    xt[ROWS] = np.float16(1.0)
    return xt


def run(enc_x, weight, bias, trace=False, **spmd_kwargs):
    """Run on 8 NeuronCores; returns (out [B, WINDOWS] fp32, BassKernelResults)."""
    from concourse.bass_utils import run_bass_kernel_spmd

    nc = _get_nc()
    xf = np.asarray(enc_x).astype(np.float16).reshape(NCORES, NWIN, K)
    wf = np.asarray(weight).astype(np.float16).reshape(K)
    bf = float(np.asarray(bias).reshape(-1)[0])

    sm = np.zeros((128 * NBLK, G), dtype=np.float16)
    r = np.arange(ROWS)
    sm[r, r // K] = wf[r % K]
    sm[ROWS, :] = np.float16(bf)

    in_maps = [{"x": _prep_core(xf[i]), "s": sm} for i in range(NCORES)]
    res = run_bass_kernel_spmd(
        nc, in_maps, list(range(NCORES)), trace=trace, **spmd_kwargs
    )
    out = np.stack(
        [res.results[i]["out"].astype(np.float32).T.reshape(NWIN) for i in range(NCORES)],
        axis=0,
    )
    return out.reshape(B, WINDOWS), res


def kernel(enc_x, weight, bias, windows_nb=None):
    out, _ = run(enc_x, weight, bias)
    return out


# revision 6
# speedup vs baseline: 1.0106x; 1.0106x over previous
"""Trainium2 Bass kernel for im2col Conv2d dot-product, PE (tensor engine)
version: out[b, n] = <enc_x[b, n, :], w_flat> + bias.

Data-parallel over batch: 8 batches per NeuronCore x 8 cores.

The host casts to fp16 (tolerance 2e-2; fp16 keeps rel err ~1e-4, and PE
accumulates in fp32 PSUM) and lays x out k-major: per core the flat
window stream [401408*49] is reshaped to [12544, 1568] and transposed to
[1568, 12544], so column c holds the 1568 contiguous values of windows
32c..32c+31; a 1569th row of ones carries the bias. The stationary
matrix S [1569, 32] is block-diagonal w (S[r, r//49] = w[r%49], ones row
= bias), split into 13 partition blocks of <=128 rows. Each output chunk
[32, 448] is produced by 13 accumulating matmuls (start on h=0, stop on
h=12), evacuated PSUM->SBUF by the ScalarE, and DMAed out as [32, 12544]
which the host transposes back to window order.

PE streams 448 cols/matmul at ~1 cycle/col (fp16) -> ~70-90 us/core of
matmul; DVE and GpSimd are idle; the kernel is DMA-bound (~39.7 MB/core
at ~340 GB/s -> ~120 us).
"""

from contextlib import ExitStack

import numpy as np

import concourse.bass as bass
import concourse.tile as tile
from concourse import mybir

B = 64
WINDOWS = 50176
K = 49
NCORES = 8
BPC = B // NCORES            # batches per core
NWIN = BPC * WINDOWS         # 401408 windows per core
G = 32                       # windows per column
ROWS = G * K                 # 1568 data rows
NBLK = (ROWS + 1 + 127) // 128   # 13 partition blocks (incl. ones row)
COLS = NWIN // G             # 12544 columns per core

CHUNK = 448                  # columns per PSUM chunk (32*448*4 = 1792B/bank)
# Variable column-tile schedule: big tiles early (fat DMAs while the
# pipeline is deep), small final tile so the post-last-DMA drain
# (h=12 matmul + PSUM evacuation + out-DMA) is short.
CTS = [3136, 3136, 3136, 1792, 896, 448]
assert sum(CTS) == COLS and all(c % CHUNK == 0 for c in CTS)
CTMAX = max(CTS)

FP32 = mybir.dt.float32
FP16 = mybir.dt.float16

_NC = None


def _build_nc():
    nc = bass.Bass(trn_type="TRN2", debug=False, num_devices=NCORES)

    x = nc.dram_tensor("x", [ROWS + 1, COLS], FP16, kind="ExternalInput").ap()
    # padded to 128*NBLK rows so one DMA loads all stationary blocks
    s = nc.dram_tensor("s", [128 * NBLK, G], FP16, kind="ExternalInput").ap()
    out = nc.dram_tensor("out", [G, COLS], FP16, kind="ExternalOutput").ap()

    blk_rows = [min(128, ROWS + 1 - 128 * h) for h in range(NBLK)]

    with tile.TileContext(nc) as tc, ExitStack() as ctx:
        consts = ctx.enter_context(tc.tile_pool(name="consts", bufs=1))
        xpool = ctx.enter_context(tc.tile_pool(name="x", bufs=2))
        apool = ctx.enter_context(tc.tile_pool(name="a", bufs=2))
        ppool = ctx.enter_context(tc.psum_pool(name="p", bufs=8))

        # all stationary blocks in one tile [128, NBLK, G], one DMA
        sblk = consts.tile([128, NBLK, G], FP16, name="sblk")
        nc.scalar.dma_start(
            out=sblk[:],
            in_=bass.AP(
                tensor=s.tensor,
                offset=s.offset,
                ap=[[G, 128], [128 * G, NBLK], [1, G]],
            ),
        )
        sts = [sblk[:][0 : blk_rows[h], h, :] for h in range(NBLK)]

        c0 = 0
        for ct, CT in enumerate(CTS):
            nchunk = CT // CHUNK
            # fixed-shape ring buffers; only the first CT columns are used
            xb = xpool.tile([128, NBLK, CTMAX], FP16, tag="xb", name=f"xb{ct}")
            xv = xb[:]
            for h in range(NBLK):
                src = bass.AP(
                    tensor=x.tensor,
                    offset=x.offset + (128 * h) * COLS + c0,
                    ap=[[COLS, blk_rows[h]], [1, CT]],
                )
                nc.sync.dma_start(out=xv[0 : blk_rows[h], h, 0:CT], in_=src)

            acc = apool.tile(
                [G, CTMAX // CHUNK, CHUNK], FP16, tag="acc", name=f"acc{ct}"
            )
            # h-outer: consecutive matmuls share the stationary block, so
            # walrus can skip redundant LDWEIGHTS; chunks accumulate in
            # parallel PSUM banks (groups interleave -> skip_group_check).
            pts = [
                ppool.tile([G, CHUNK], FP32, tag="pt", name=f"pt{ct}_{ci}")
                for ci in range(nchunk)
            ]
            for h in range(NBLK):
                for ci in range(nchunk):
                    nc.tensor.matmul(
                        pts[ci][:],
                        sts[h],
                        xv[0 : blk_rows[h], h, ci * CHUNK : (ci + 1) * CHUNK],
                        start=(h == 0),
                        stop=(h == NBLK - 1),
                        skip_group_check=True,
                    )
            for ci in range(nchunk):
                nc.scalar.copy(out=acc[:, ci, :], in_=pts[ci][:])

            dst = bass.AP(
                tensor=out.tensor,
                offset=out.offset + c0,
                ap=[[COLS, G], [1, CT]],
            )
            nc.scalar.dma_start(
                out=dst,
                in_=acc[:, 0:nchunk, :].rearrange("g c k -> g (c k)"),
            )
            c0 += CT
        assert c0 == COLS

    return nc


def _split_ctrl_waits(nc, max_waits=1):
    """Work around a walrus codegen limit on this build: instructions accept
    only one sync-wait command. Hoist extra waits onto dedicated no-op
    instructions inserted just before, preserving per-engine order."""
    from concourse import mybir

    for f in nc.m.functions:
        for blk in f.blocks:
            insts = blk.instructions
            i = 0
            while i < len(insts):
                ins = insts[i]
                if (
                    ins.sync_info is not None
                    and len(ins.sync_info.on_wait) > max_waits
                ):
                    waits = list(ins.sync_info.on_wait)
                    keep, extra = waits[:max_waits], waits[max_waits:]
                    ins.sync_info.on_wait = keep
                    for j, wchunk in enumerate(extra):
                        nop = mybir.InstNoOp(
                            name=f"{ins.name}-wsplit{j}",
                            sync_info=mybir.SyncInfo(on_wait=[wchunk], on_update=[]),
                            bass_nofuse=True,
                            engine=ins.engine,
                        )
                        nc.register_instruction(nop, overwrite=True)
                        insts.insert(i, nop)
                        i += 1
                i += 1


def _get_nc():
    global _NC
    if _NC is None:
        _NC = _build_nc()
        _split_ctrl_waits(_NC)
    return _NC


def _prep_core(xc16):
    """[NWIN, K] fp16 -> [ROWS+1, COLS] fp16 k-major layout + ones row."""
    xt = np.empty((ROWS + 1, COLS), dtype=np.float16)
    xt[:ROWS] = xc16.reshape(COLS, ROWS).T
    xt[ROWS] = np.float16(1.0)
    return xt


def run(enc_x, weight, bias, trace=False, **spmd_kwargs):
    """Run on 8 NeuronCores; returns (out [B, WINDOWS] fp32, BassKernelResults)."""
    from concourse.bass_utils import run_bass_kernel_spmd

    nc = _get_nc()
    xf = np.asarray(enc_x).astype(np.float16).reshape(NCORES, NWIN, K)
    wf = np.asarray(weight).astype(np.float16).reshape(K)
    bf = float(np.asarray(bias).reshape(-1)[0])

    sm = np.zeros((128 * NBLK, G), dtype=np.float16)
    r = np.arange(ROWS)
    sm[r, r // K] = wf[r % K]
    sm[ROWS, :] = np.float16(bf)

    in_maps = [{"x": _prep_core(xf[i]), "s": sm} for i in range(NCORES)]
    res = run_bass_kernel_spmd(
        nc, in_maps, list(range(NCORES)), trace=trace, **spmd_kwargs
    )
    out = np.stack(
        [res.results[i]["out"].astype(np.float32).T.reshape(NWIN) for i in range(NCORES)],
        axis=0,
    )
    return out.reshape(B, WINDOWS), res


def kernel(enc_x, weight, bias, windows_nb=None):
    out, _ = run(enc_x, weight, bias)
    return out
